# revision 5
# baseline (speedup 1.0000x reference)
"""Trainium2 Bass kernel for ContentPopularityJointAttention.

Computes, for each batch row r:
    mp     = concat(m[r], p[r])            # (50, 512)
    hidden = tanh(mp @ Wu)                 # (50, 512)
    s      = hidden @ b                    # (50,)
    u[r]   = (sum_n s_n * m[r,n]) / (sum_n s_n)   # (256,)

Sharding: pure data parallel over the batch dim across 8 NeuronCores.

Per-core dataflow (512 rows = 25600 tokens, 128-token chunks, fp16 data
path):
  1. Host converts m,p to fp16.  Per 10-chunk macroblock, three HWDGE
     DMAs (issued on SP): XBAR dma-transpose of m and p -> feature-major
     [128d, 2, 1280tok] fp16 tiles, plus token-major m [128, 10, 256]
     for pooling.  Batching DMAs 10 chunks at a time amortizes the
     ~650ns/instruction DGE cost.
  2. Hidden matmul: 4 fp16 matmuls per chunk (mpT d-tile stationary,
     Wu d-tile [128,512] moving) -> PSUM [128tok, 512] fp32.  fp16 runs
     1 cycle/row vs fp32's 4; the precision loss is handled by the
     host-side fixup (step 6).
  3. ACT tanh on a 2-chunk batch [128, 1024] -> fp16 SBUF.
  4. DVE scalar_tensor_tensor: s = sum_k tanh*b in one fused op
     (fp32 accumulate) -> per-token scores [128,1].
  5. Pooling per chunk: blk16 = mask*s (fp16) -> one fp16 matmul
     accumulates C = sum s*m into a 64-row group PSUM [64, 256];
     blk32 = mask*s (fp32) with a [128,1] ones rhs accumulates
     S = sum s exactly in fp32 (out free = 1, so 4 cycles).  Group ends:
     DVE reciprocal + scale -> u rows + 1/S, one DMA out [64, 257].
  6. Host fixup: the sum-normalization a = s/S amplifies error by
     ~|C|/S^2; rows with |S| < 20 (~55 of 4096) are recomputed exactly
     on host in fp64.  Device-side fp16 error (ds ~ 0.05 abs) is
     harmless for all other rows.
"""

import numpy as np
from contextlib import ExitStack

import concourse.bass as bass
import concourse.bacc as bacc
import concourse.tile as tile
from concourse import mybir
from concourse.bass_utils import run_bass_kernel_spmd

N_CORES = 8
B_FULL, N_TOK, MD, PD = 4096, 50, 256, 256
D = MD + PD          # 512 contraction dim
K = 512              # hidden dim
CHUNK = 128          # tokens per chunk (partition dim)
GROUP_ROWS = 64      # batch rows per pooling PSUM accumulation group
GROUP_CHUNKS = GROUP_ROWS * N_TOK // CHUNK   # 25 (tiles the group exactly)
GDMA = 10            # chunks per batched DMA macroblock
S_FIX_THRESH = 20.0  # host fixup threshold on |S|

f32 = mybir.dt.float32
f16 = mybir.dt.float16

_prog_cache: dict = {}


def build_program(b_shard: int):
    """Build the single-core Bass program (SPMD: same program, all cores)."""
    tokens = b_shard * N_TOK
    nchunks = tokens // CHUNK
    assert b_shard % GROUP_ROWS == 0

    nc = bacc.Bacc("TRN2", target_bir_lowering=False, debug=False,
                   num_devices=N_CORES)

    m_d = nc.dram_tensor("m", [tokens, MD], f16, kind="ExternalInput").ap()
    p_d = nc.dram_tensor("p", [tokens, PD], f16, kind="ExternalInput").ap()
    wu_d = nc.dram_tensor("wu", [128, 4, K], f16, kind="ExternalInput").ap()
    brep_d = nc.dram_tensor("brep", [128, K], f16, kind="ExternalInput").ap()
    mask16_d = nc.dram_tensor("mask16", [128, GROUP_CHUNKS, GROUP_ROWS], f16,
                              kind="ExternalInput").ap()
    mask32_d = nc.dram_tensor("mask32", [128, GROUP_CHUNKS, GROUP_ROWS], f32,
                              kind="ExternalInput").ap()
    ones_d = nc.dram_tensor("ones", [128, 1], f32, kind="ExternalInput").ap()
    u_d = nc.dram_tensor("u", [b_shard, MD + 1], f32, kind="ExternalOutput").ap()

    with tile.TileContext(nc) as tc, ExitStack() as ctx:
        singles = ctx.enter_context(tc.tile_pool(name="singles", bufs=1))
        io_t = ctx.enter_context(tc.tile_pool(name="ioT", bufs=2))
        io_m = ctx.enter_context(tc.tile_pool(name="ioM", bufs=2))
        io_u = ctx.enter_context(tc.tile_pool(name="ioU", bufs=2))
        work = ctx.enter_context(tc.tile_pool(name="work", bufs=3))
        psum_h = ctx.enter_context(tc.tile_pool(name="psumH", bufs=2, space="PSUM"))
        psum_u = ctx.enter_context(tc.tile_pool(name="psumU", bufs=2, space="PSUM"))
        psum_s = ctx.enter_context(tc.tile_pool(name="psumS", bufs=2, space="PSUM"))

        wu_sb = singles.tile([128, 4, K], f16)
        nc.sync.dma_start(out=wu_sb[:], in_=wu_d)
        brep_sb = singles.tile([128, K], f16)
        nc.sync.dma_start(out=brep_sb[:], in_=brep_d)
        mask16_sb = singles.tile([128, GROUP_CHUNKS, GROUP_ROWS], f16)
        nc.sync.dma_start(out=mask16_sb[:], in_=mask16_d)
        mask32_sb = singles.tile([128, GROUP_CHUNKS, GROUP_ROWS], f32)
        nc.sync.dma_start(out=mask32_sb[:], in_=mask32_d)
        ones_sb = singles.tile([128, 1], f32)
        nc.sync.dma_start(out=ones_sb[:], in_=ones_d)
        scratch = singles.tile([128, K], f16)   # dead product of fused score op

        mT = pT = mtok = None
        hid2 = None
        pool_ps = None
        mb0 = 0  # first chunk of current macroblock

        for c in range(nchunks):
            blk_i, l = divmod(c, GDMA)
            if l == 0:
                nb = min(GDMA, nchunks - blk_i * GDMA)
                mb0 = blk_i * GDMA
                t0 = mb0 * CHUNK
                span = nb * CHUNK
                mT = io_t.tile([128, 2, span], f16)
                nc.sync.dma_start_transpose(out=mT[:], in_=m_d[t0:t0 + span, :])
                pT = io_t.tile([128, 2, span], f16)
                nc.sync.dma_start_transpose(out=pT[:], in_=p_d[t0:t0 + span, :])
                mtok = io_m.tile([128, nb, MD], f16)
                nc.sync.dma_start(
                    out=mtok[:],
                    in_=m_d[t0:t0 + span, :].rearrange("(g p) d -> p g d", p=128))

            par = c % 2
            if par == 0:
                hid2 = psum_h.tile([128, 2, K], f32)
            # hidden = mp @ Wu for this chunk (tokens l*128 .. within macroblock)
            for j in range(4):
                src = mT if j < 2 else pT
                nc.tensor.matmul(
                    hid2[:, par, :],
                    lhsT=src[:, j % 2, l * CHUNK:(l + 1) * CHUNK],
                    rhs=wu_sb[:, j, :],
                    start=(j == 0),
                    stop=(j == 3),
                )

            if par == 1 or c == nchunks - 1:
                npair = par + 1
                th2 = work.tile([128, 2, K], f16)
                nc.scalar.activation(out=th2[:, 0:npair, :], in_=hid2[:, 0:npair, :],
                                     func=mybir.ActivationFunctionType.Tanh)
                for pp in range(npair):
                    cc = c - par + pp
                    gg, cci = divmod(cc, GROUP_CHUNKS)
                    if cci == 0:
                        pool_ps = psum_u.tile([GROUP_ROWS, MD], f32)
                        s_ps = psum_s.tile([GROUP_ROWS, 1], f32)
                    s_t = work.tile([128, 1], f32)
                    nc.vector.scalar_tensor_tensor(
                        out=scratch[:], in0=th2[:, pp, :], scalar=1.0,
                        in1=brep_sb[:], op0=mybir.AluOpType.mult,
                        op1=mybir.AluOpType.mult, accum_out=s_t[:])
                    blk16 = work.tile([128, GROUP_ROWS], f16)
                    nc.vector.tensor_scalar_mul(blk16[:], mask16_sb[:, cci, :], s_t[:])
                    blk32 = work.tile([128, GROUP_ROWS], f32)
                    nc.vector.tensor_scalar_mul(blk32[:], mask32_sb[:, cci, :], s_t[:])
                    ll = cc - mb0
                    nc.tensor.matmul(
                        pool_ps[:],
                        lhsT=blk16[:],
                        rhs=mtok[:, ll, :],
                        start=(cci == 0),
                        stop=(cci == GROUP_CHUNKS - 1),
                    )
                    nc.tensor.matmul(
                        s_ps[:],
                        lhsT=blk32[:],
                        rhs=ones_sb[:],
                        start=(cci == 0),
                        stop=(cci == GROUP_CHUNKS - 1),
                    )
                    if cci == GROUP_CHUNKS - 1:
                        u_sb = io_u.tile([GROUP_ROWS, MD + 1], f32)
                        nc.vector.reciprocal(u_sb[:, MD:MD + 1], s_ps[:])
                        nc.vector.tensor_scalar_mul(u_sb[:, 0:MD],
                                                    pool_ps[:],
                                                    u_sb[:, MD:MD + 1])
                        nc.sync.dma_start(
                            out=u_d[gg * GROUP_ROWS:(gg + 1) * GROUP_ROWS, :],
                            in_=u_sb[:])

    nc.compile()
    return nc


def host_constants(Wu: np.ndarray, b: np.ndarray):
    Wu = np.asarray(Wu, np.float32)
    b = np.asarray(b, np.float32)
    # [d, k] -> [d%128, d//128, k]
    wu = np.ascontiguousarray(
        Wu.astype(np.float16).reshape(4, 128, K).transpose(1, 0, 2))
    brep = np.ascontiguousarray(
        np.broadcast_to(b.astype(np.float16), (128, K)))
    tp = np.arange(128)[:, None, None]
    ll = np.arange(GROUP_CHUNKS)[None, :, None]
    rr = np.arange(GROUP_ROWS)[None, None, :]
    mask = ((CHUNK * ll + tp) // N_TOK) == rr
    return {
        "wu": wu,
        "brep": brep,
        "mask16": mask.astype(np.float16),
        "mask32": mask.astype(np.float32),
        "ones": np.ones((128, 1), np.float32),
    }


def get_program(b_shard: int):
    if b_shard not in _prog_cache:
        _prog_cache[b_shard] = build_program(b_shard)
    return _prog_cache[b_shard]


def kernel(m: np.ndarray, p: np.ndarray, Wu: np.ndarray, b: np.ndarray
           ) -> np.ndarray:
    m = np.ascontiguousarray(np.asarray(m, np.float32))
    p = np.ascontiguousarray(np.asarray(p, np.float32))
    B = m.shape[0]
    assert B % N_CORES == 0
    b_shard = B // N_CORES

    nc = get_program(b_shard)
    consts = host_constants(Wu, b)

    mh = m.reshape(B * N_TOK, MD).astype(np.float16)
    ph = p.reshape(B * N_TOK, PD).astype(np.float16)
    tok_sh = b_shard * N_TOK
    in_maps = []
    for c in range(N_CORES):
        in_maps.append({
            "m": mh[c * tok_sh:(c + 1) * tok_sh],
            "p": ph[c * tok_sh:(c + 1) * tok_sh],
            **consts,
        })
    res = run_bass_kernel_spmd(nc, in_maps, list(range(N_CORES)))
    u257 = np.concatenate([res.results[c]["u"] for c in range(N_CORES)], axis=0)
    u = np.ascontiguousarray(u257[:, 0:MD], dtype=np.float32)

    # Host fixup: rows where |S| is small amplify device fp16 error via
    # a = s/S; recompute those exactly in fp64 from the original inputs.
    with np.errstate(divide="ignore", over="ignore", invalid="ignore"):
        S_dev = 1.0 / u257[:, MD]
    bad = ~np.isfinite(S_dev) | (np.abs(S_dev) < S_FIX_THRESH)
    bad |= ~np.isfinite(u).all(axis=1)
    if bad.any():
        idx = np.where(bad)[0]
        Wu64 = np.asarray(Wu, np.float64)
        b64 = np.asarray(b, np.float64)
        mp = np.concatenate([m[idx], p[idx]], axis=2).reshape(-1, D)
        th = np.tanh(mp.astype(np.float64) @ Wu64)
        s = (th @ b64).reshape(len(idx), N_TOK)
        S = s.sum(axis=1)
        C = np.einsum("bn,bnd->bd", s, m[idx].astype(np.float64))
        u[idx] = (C / S[:, None]).astype(np.float32)
    return u


# revision 8
# speedup vs baseline: 1.0860x; 1.0860x over previous
"""Trainium2 Bass kernel for ContentPopularityJointAttention.

Computes, for each batch row r:
    mp     = concat(m[r], p[r])            # (50, 512)
    hidden = tanh(mp @ Wu)                 # (50, 512)
    s      = hidden @ b                    # (50,)
    u[r]   = (sum_n s_n * m[r,n]) / (sum_n s_n)   # (256,)

Sharding: pure data parallel over the batch dim across 8 NeuronCores.

Per-core dataflow (512 rows = 25600 tokens, 128-token chunks, fp16 data
path):
  1. Host converts m,p to fp16.  Per macroblock (4,6,10,10,... chunks;
     small first blocks so PE starts ~3us in), three HWDGE DMAs on SP:
     XBAR dma-transpose of m and p -> feature-major [128d, 2, span] fp16,
     plus token-major m [128, nb, 256] for pooling.  Batching amortizes
     the ~650ns/instruction DGE cost.
  2. Hidden matmul: 4 fp16 matmuls per chunk (mpT d-tile stationary,
     Wu d-tile [128,512] moving) -> PSUM [128tok, 512] fp32.  fp16 runs
     1 cycle/row vs fp32's 4; the precision loss is handled by the
     host-side fixup (step 6).
  3. ACT tanh on a 2-chunk batch [128, 1024] -> fp16 SBUF.
  4. DVE scalar_tensor_tensor: s = sum_k tanh*b in one fused op
     (fp32 accumulate) -> per-token scores [128,1].
  5. Pooling per chunk, transposed so the 64-row blk is the MOVING
     operand (2 matmuls of out-free 64 = 128 PE cycles instead of 256):
     CT[d, row] += mtok_half^T @ (mask*s fp16); S[row] += (mask*s fp32)
     @ ones (exact fp32, out free 1 = 4 cycles).  Group end: ACT copies
     CT/S PSUM->SBUF, DMA out; the u = C/S division happens on host.
  6. Host fixup: the sum-normalization a = s/S amplifies error by
     ~|C|/S^2; rows with |S| < 20 (~55 of 4096) are recomputed exactly
     on host in fp64.  Device-side fp16 error (ds ~ 0.05 abs) is
     harmless for all other rows.
"""

import numpy as np
from contextlib import ExitStack

import concourse.bass as bass
import concourse.bacc as bacc
import concourse.tile as tile
from concourse import mybir
from concourse.bass_utils import run_bass_kernel_spmd

N_CORES = 8
B_FULL, N_TOK, MD, PD = 4096, 50, 256, 256
D = MD + PD          # 512 contraction dim
K = 512              # hidden dim
CHUNK = 128          # tokens per chunk (partition dim)
GROUP_ROWS = 64      # batch rows per pooling PSUM accumulation group
GROUP_CHUNKS = GROUP_ROWS * N_TOK // CHUNK   # 25 (tiles the group exactly)
S_FIX_THRESH = 20.0  # host fixup threshold on |S|

f32 = mybir.dt.float32
f16 = mybir.dt.float16

_prog_cache: dict = {}


def _macroblocks(nchunks: int):
    """(start, nb) DMA macroblocks: small leading blocks to start PE early."""
    sizes = []
    for sz in (4, 6):
        if sum(sizes) + sz <= nchunks:
            sizes.append(sz)
    while (rem := nchunks - sum(sizes)) > 0:
        sizes.append(min(10, rem))
    out, c0 = [], 0
    for sz in sizes:
        out.append((c0, sz))
        c0 += sz
    return out


def build_program(b_shard: int):
    """Build the single-core Bass program (SPMD: same program, all cores)."""
    tokens = b_shard * N_TOK
    nchunks = tokens // CHUNK
    n_groups = b_shard // GROUP_ROWS
    assert b_shard % GROUP_ROWS == 0
    mblocks = dict(_macroblocks(nchunks))

    nc = bacc.Bacc("TRN2", target_bir_lowering=False, debug=False,
                   num_devices=N_CORES)

    m_d = nc.dram_tensor("m", [tokens, MD], f16, kind="ExternalInput").ap()
    p_d = nc.dram_tensor("p", [tokens, PD], f16, kind="ExternalInput").ap()
    wu_d = nc.dram_tensor("wu", [128, 4, K], f16, kind="ExternalInput").ap()
    brep_d = nc.dram_tensor("brep", [128, K], f16, kind="ExternalInput").ap()
    mask16_d = nc.dram_tensor("mask16", [128, GROUP_CHUNKS, GROUP_ROWS], f16,
                              kind="ExternalInput").ap()
    ct_d = nc.dram_tensor("ct", [n_groups, 128, 2, GROUP_ROWS], f32,
                          kind="ExternalOutput").ap()
    s_d = nc.dram_tensor("sv", [b_shard, 1], f32, kind="ExternalOutput").ap()

    with tile.TileContext(nc) as tc, ExitStack() as ctx:
        singles = ctx.enter_context(tc.tile_pool(name="singles", bufs=1))
        io_t = ctx.enter_context(tc.tile_pool(name="ioT", bufs=2))
        io_m = ctx.enter_context(tc.tile_pool(name="ioM", bufs=2))
        io_u = ctx.enter_context(tc.tile_pool(name="ioU", bufs=2))
        work = ctx.enter_context(tc.tile_pool(name="work", bufs=3))
        psum_h = ctx.enter_context(tc.tile_pool(name="psumH", bufs=2, space="PSUM"))
        psum_u = ctx.enter_context(tc.tile_pool(name="psumU", bufs=2, space="PSUM"))
        psum_s = ctx.enter_context(tc.tile_pool(name="psumS", bufs=2, space="PSUM"))

        # singles allocated up front; DMAs issued inside the loop (first
        # macroblock's data DMAs go out first so PE starts early)
        wu_sb = singles.tile([128, 4, K], f16)
        brep_sb = singles.tile([128, K], f16)
        mask16_sb = singles.tile([128, GROUP_CHUNKS, GROUP_ROWS], f16)
        mask32_sb = singles.tile([128, GROUP_CHUNKS, GROUP_ROWS], f32)
        ones_sb = singles.tile([128, 1], f32)
        scratch = singles.tile([128, K], f16)   # dead product of fused score op

        mT = pT = mtok = None
        ct_ps = s_ps = None
        mb0 = 0

        for c in range(nchunks):
            if c in mblocks:
                nb = mblocks[c]
                mb0 = c
                t0 = c * CHUNK
                span = nb * CHUNK
                mT = io_t.tile([128, 2, span], f16)
                nc.sync.dma_start_transpose(out=mT[:], in_=m_d[t0:t0 + span, :])
                pT = io_t.tile([128, 2, span], f16)
                nc.sync.dma_start_transpose(out=pT[:], in_=p_d[t0:t0 + span, :])
                mtok = io_m.tile([128, nb, MD], f16)
                nc.sync.dma_start(
                    out=mtok[:],
                    in_=m_d[t0:t0 + span, :].rearrange("(g p) d -> p g d", p=128))
                if c == 0:
                    nc.sync.dma_start(out=wu_sb[:], in_=wu_d)
                    nc.sync.dma_start(out=brep_sb[:], in_=brep_d)
                    nc.sync.dma_start(out=mask16_sb[:], in_=mask16_d)
                    nc.vector.memset(ones_sb[:], 1.0)
                    # fp32 masks derived on the otherwise-idle Pool engine
                    nc.gpsimd.tensor_copy(out=mask32_sb[:], in_=mask16_sb[:])

            l = c - mb0
            gg, cci = divmod(c, GROUP_CHUNKS)
            if cci == 0:
                # halves padded to separate PSUM banks (independent
                # accumulation groups must not share a zero region)
                ct_ps = psum_u.tile([128, 2, K], f32)
                s_ps = psum_s.tile([GROUP_ROWS, 1], f32)
            hid = psum_h.tile([128, K], f32)
            # hidden = mp @ Wu for this chunk
            for j in range(4):
                src = mT if j < 2 else pT
                nc.tensor.matmul(
                    hid[:],
                    lhsT=src[:, j % 2, l * CHUNK:(l + 1) * CHUNK],
                    rhs=wu_sb[:, j, :],
                    start=(j == 0),
                    stop=(j == 3),
                )
            th = work.tile([128, K], f16)
            nc.scalar.activation(out=th[:], in_=hid[:],
                                 func=mybir.ActivationFunctionType.Tanh)
            s_t = work.tile([128, 1], f32)
            nc.vector.scalar_tensor_tensor(
                out=scratch[:], in0=th[:], scalar=1.0,
                in1=brep_sb[:], op0=mybir.AluOpType.mult,
                op1=mybir.AluOpType.mult, accum_out=s_t[:])
            blk16 = work.tile([128, GROUP_ROWS], f16)
            nc.vector.tensor_scalar_mul(blk16[:], mask16_sb[:, cci, :], s_t[:])
            blk32 = work.tile([128, GROUP_ROWS], f32)
            nc.vector.tensor_scalar_mul(blk32[:], mask32_sb[:, cci, :], s_t[:])
            for h in range(2):
                nc.tensor.matmul(
                    ct_ps[:, h, 0:GROUP_ROWS],
                    lhsT=mtok[:, l, h * 128:(h + 1) * 128],
                    rhs=blk16[:],
                    start=(cci == 0),
                    stop=(cci == GROUP_CHUNKS - 1),
                )
            nc.tensor.matmul(
                s_ps[:],
                lhsT=blk32[:],
                rhs=ones_sb[:],
                start=(cci == 0),
                stop=(cci == GROUP_CHUNKS - 1),
            )
            if cci == GROUP_CHUNKS - 1:
                ct_sb = io_u.tile([128, 2, GROUP_ROWS], f32)
                nc.scalar.copy(out=ct_sb[:], in_=ct_ps[:, :, 0:GROUP_ROWS])
                s_sb = io_u.tile([GROUP_ROWS, 1], f32)
                nc.scalar.copy(out=s_sb[:], in_=s_ps[:])
                nc.sync.dma_start(out=ct_d[gg], in_=ct_sb[:])
                nc.sync.dma_start(
                    out=s_d[gg * GROUP_ROWS:(gg + 1) * GROUP_ROWS, :],
                    in_=s_sb[:])

    nc.compile()
    return nc


def host_constants(Wu: np.ndarray, b: np.ndarray):
    Wu = np.asarray(Wu, np.float32)
    b = np.asarray(b, np.float32)
    # [d, k] -> [d%128, d//128, k]
    wu = np.ascontiguousarray(
        Wu.astype(np.float16).reshape(4, 128, K).transpose(1, 0, 2))
    brep = np.ascontiguousarray(
        np.broadcast_to(b.astype(np.float16), (128, K)))
    tp = np.arange(128)[:, None, None]
    ll = np.arange(GROUP_CHUNKS)[None, :, None]
    rr = np.arange(GROUP_ROWS)[None, None, :]
    mask = ((CHUNK * ll + tp) // N_TOK) == rr
    return {
        "wu": wu,
        "brep": brep,
        "mask16": mask.astype(np.float16),
    }


def get_program(b_shard: int):
    if b_shard not in _prog_cache:
        _prog_cache[b_shard] = build_program(b_shard)
    return _prog_cache[b_shard]


def kernel(m: np.ndarray, p: np.ndarray, Wu: np.ndarray, b: np.ndarray
           ) -> np.ndarray:
    m = np.ascontiguousarray(np.asarray(m, np.float32))
    p = np.ascontiguousarray(np.asarray(p, np.float32))
    B = m.shape[0]
    assert B % N_CORES == 0
    b_shard = B // N_CORES

    nc = get_program(b_shard)
    consts = host_constants(Wu, b)

    mh = m.reshape(B * N_TOK, MD).astype(np.float16)
    ph = p.reshape(B * N_TOK, PD).astype(np.float16)
    tok_sh = b_shard * N_TOK
    in_maps = []
    for c in range(N_CORES):
        in_maps.append({
            "m": mh[c * tok_sh:(c + 1) * tok_sh],
            "p": ph[c * tok_sh:(c + 1) * tok_sh],
            **consts,
        })
    res = run_bass_kernel_spmd(nc, in_maps, list(range(N_CORES)))
    # ct: [n_groups, 128, 2, 64] per core; C[row, h*128+dp] = ct[g, dp, h, r]
    ct = np.concatenate([res.results[c]["ct"] for c in range(N_CORES)], axis=0)
    S_dev = np.concatenate(
        [res.results[c]["sv"] for c in range(N_CORES)], axis=0)[:, 0]
    C = ct.transpose(0, 3, 2, 1).reshape(B, MD)  # [G,dp,h,r]->[G,r,h,dp]
    with np.errstate(divide="ignore", over="ignore", invalid="ignore"):
        u = (C / S_dev[:, None]).astype(np.float32)

    # Host fixup: rows where |S| is small amplify device fp16 error via
    # a = s/S; recompute those exactly in fp64 from the original inputs.
    bad = ~np.isfinite(S_dev) | (np.abs(S_dev) < S_FIX_THRESH)
    bad |= ~np.isfinite(u).all(axis=1)
    if bad.any():
        idx = np.where(bad)[0]
        Wu64 = np.asarray(Wu, np.float64)
        b64 = np.asarray(b, np.float64)
        mp = np.concatenate([m[idx], p[idx]], axis=2).reshape(-1, D)
        th = np.tanh(mp.astype(np.float64) @ Wu64)
        s = (th @ b64).reshape(len(idx), N_TOK)
        S = s.sum(axis=1)
        Cx = np.einsum("bn,bnd->bd", s, m[idx].astype(np.float64))
        u[idx] = (Cx / S[:, None]).astype(np.float32)
    return u


# revision 9
# speedup vs baseline: 1.2805x; 1.1792x over previous
"""Trainium2 Bass kernel for ContentPopularityJointAttention.

Computes, for each batch row r:
    mp     = concat(m[r], p[r])            # (50, 512)
    hidden = tanh(mp @ Wu)                 # (50, 512)
    s      = hidden @ b                    # (50,)
    u[r]   = (sum_n s_n * m[r,n]) / (sum_n s_n)   # (256,)

Sharding: pure data parallel over the batch dim across 8 NeuronCores.

Per-core dataflow (512 rows = 25600 tokens, 128-token chunks):
  1. Host splits mp = concat(m,p) into fp8e4m3 hi + lo (hi = e4m3(mp),
     lo = e4m3(mp - hi); together ~9 mantissa bits) and pre-transposes
     both to the DoubleRow feature-major layout [128p, 2pair, 2kt, tok].
     Per macroblock (4,6,10,... chunks), three batched HWDGE DMAs on SP:
     hi, lo, and token-major fp16 m for pooling.
  2. Hidden matmul: 6 fp8 DoubleRow matmuls per chunk (3 passes
     hi*hi + lo*hi + hi*lo, each 2 pair-matmuls with 256-contraction)
     at 0.5 cycles/row -> PSUM [128tok, 512] fp32.  1536 PE cycles vs
     fp16's 2048; dropped lo*lo term ~2^-9 relative.
  3. ACT tanh -> fp16 SBUF.  4. DVE fused scalar_tensor_tensor:
     s = sum_k tanh*b (fp32 accumulate) -> [128,1].
  5. Pooling per chunk, transposed so the 64-row blk is the MOVING
     operand (2 matmuls of out-free 64 = 128 PE cycles):
     CT[d, row] += mtok_half^T @ (mask*s fp16); S[row] += (mask*s fp32)
     @ ones (exact fp32, 4 cycles).  Group end: ACT copies CT/S
     PSUM->SBUF, DMA out; u = C/S division happens on host.
  6. Host fixup: the sum-normalization a = s/S amplifies error by
     ~|C|/S^2; rows with |S| < 50 (~130 of 4096) are recomputed exactly
     on host in fp64.  Device-side fp8-split error (ds ~ 0.2 abs) is
     harmless for all other rows.
"""

import numpy as np
import ml_dtypes
from contextlib import ExitStack

import concourse.bass as bass
import concourse.bacc as bacc
import concourse.tile as tile
from concourse import mybir
from concourse.bass_utils import run_bass_kernel_spmd

N_CORES = 8
B_FULL, N_TOK, MD, PD = 4096, 50, 256, 256
D = MD + PD          # 512 contraction dim
K = 512              # hidden dim
CHUNK = 128          # tokens per chunk (partition dim)
GROUP_ROWS = 64      # batch rows per pooling PSUM accumulation group
GROUP_CHUNKS = GROUP_ROWS * N_TOK // CHUNK   # 25 (tiles the group exactly)
S_FIX_THRESH = 50.0  # host fixup threshold on |S|

f32 = mybir.dt.float32
f16 = mybir.dt.float16
f8 = mybir.dt.float8e4
E4 = ml_dtypes.float8_e4m3

_prog_cache: dict = {}


def _macroblocks(nchunks: int):
    """(start, nb) DMA macroblocks: small leading blocks to start PE early."""
    sizes = []
    for sz in (4, 6):
        if sum(sizes) + sz <= nchunks:
            sizes.append(sz)
    while (rem := nchunks - sum(sizes)) > 0:
        sizes.append(min(10, rem))
    out, c0 = [], 0
    for sz in sizes:
        out.append((c0, sz))
        c0 += sz
    return out


def build_program(b_shard: int):
    """Build the single-core Bass program (SPMD: same program, all cores)."""
    tokens = b_shard * N_TOK
    nchunks = tokens // CHUNK
    n_groups = b_shard // GROUP_ROWS
    assert b_shard % GROUP_ROWS == 0
    mblocks = dict(_macroblocks(nchunks))

    nc = bacc.Bacc("TRN2", target_bir_lowering=False, debug=False,
                   num_devices=N_CORES)

    # mp transposed fp8 hi/lo: [p, pair, kt, t] = mp[t, pair*256 + kt*128 + p]
    mp8h_d = nc.dram_tensor("mp8h", [128, 2, 2, tokens], f8,
                            kind="ExternalInput").ap()
    mp8l_d = nc.dram_tensor("mp8l", [128, 2, 2, tokens], f8,
                            kind="ExternalInput").ap()
    m16_d = nc.dram_tensor("m16", [tokens, MD], f16, kind="ExternalInput").ap()
    # Wu fp8 hi/lo: [p, pair, kt, k] = Wu[pair*256 + kt*128 + p, k]
    wu8h_d = nc.dram_tensor("wu8h", [128, 2, 2, K], f8, kind="ExternalInput").ap()
    wu8l_d = nc.dram_tensor("wu8l", [128, 2, 2, K], f8, kind="ExternalInput").ap()
    brep_d = nc.dram_tensor("brep", [128, K], f16, kind="ExternalInput").ap()
    mask16_d = nc.dram_tensor("mask16", [128, GROUP_CHUNKS, GROUP_ROWS], f16,
                              kind="ExternalInput").ap()
    ct_d = nc.dram_tensor("ct", [n_groups, 128, 2, GROUP_ROWS], f32,
                          kind="ExternalOutput").ap()
    s_d = nc.dram_tensor("sv", [b_shard, 1], f32, kind="ExternalOutput").ap()

    with tile.TileContext(nc) as tc, ExitStack() as ctx:
        singles = ctx.enter_context(tc.tile_pool(name="singles", bufs=1))
        io_t = ctx.enter_context(tc.tile_pool(name="ioT", bufs=2))
        io_m = ctx.enter_context(tc.tile_pool(name="ioM", bufs=2))
        io_u = ctx.enter_context(tc.tile_pool(name="ioU", bufs=2))
        work = ctx.enter_context(tc.tile_pool(name="work", bufs=3))
        psum_h = ctx.enter_context(tc.tile_pool(name="psumH", bufs=2, space="PSUM"))
        psum_u = ctx.enter_context(tc.tile_pool(name="psumU", bufs=2, space="PSUM"))
        psum_s = ctx.enter_context(tc.tile_pool(name="psumS", bufs=2, space="PSUM"))

        wu8h_sb = singles.tile([128, 2, 2, K], f8)
        wu8l_sb = singles.tile([128, 2, 2, K], f8)
        brep_sb = singles.tile([128, K], f16)
        mask16_sb = singles.tile([128, GROUP_CHUNKS, GROUP_ROWS], f16)
        mask32_sb = singles.tile([128, GROUP_CHUNKS, GROUP_ROWS], f32)
        ones_sb = singles.tile([128, 1], f32)
        scratch = singles.tile([128, K], f16)   # dead product of fused score op

        mp8h = mp8l = mtok = None
        ct_ps = s_ps = None
        mb0 = 0

        for c in range(nchunks):
            if c in mblocks:
                nb = mblocks[c]
                mb0 = c
                t0 = c * CHUNK
                span = nb * CHUNK
                mp8h = io_t.tile([128, 2, 2, span], f8)
                nc.sync.dma_start(out=mp8h[:], in_=mp8h_d[:, :, :, t0:t0 + span])
                mp8l = io_t.tile([128, 2, 2, span], f8)
                nc.sync.dma_start(out=mp8l[:], in_=mp8l_d[:, :, :, t0:t0 + span])
                mtok = io_m.tile([128, nb, MD], f16)
                nc.sync.dma_start(
                    out=mtok[:],
                    in_=m16_d[t0:t0 + span, :].rearrange("(g p) d -> p g d", p=128))
                if c == 0:
                    nc.sync.dma_start(out=wu8h_sb[:], in_=wu8h_d)
                    nc.sync.dma_start(out=wu8l_sb[:], in_=wu8l_d)
                    nc.sync.dma_start(out=brep_sb[:], in_=brep_d)
                    nc.sync.dma_start(out=mask16_sb[:], in_=mask16_d)
                    nc.vector.memset(ones_sb[:], 1.0)
                    # fp32 masks derived on the otherwise-idle Pool engine
                    nc.gpsimd.tensor_copy(out=mask32_sb[:], in_=mask16_sb[:])

            l = c - mb0
            gg, cci = divmod(c, GROUP_CHUNKS)
            if cci == 0:
                # halves padded to separate PSUM banks (independent
                # accumulation groups must not share a zero region)
                ct_ps = psum_u.tile([128, 2, K], f32)
                s_ps = psum_s.tile([GROUP_ROWS, 1], f32)
            hid = psum_h.tile([128, K], f32)
            # hidden = mp @ Wu: 3-pass fp8 split, DoubleRow (256-contraction)
            i = 0
            for lhs8, wu8 in ((mp8h, wu8h_sb), (mp8l, wu8h_sb), (mp8h, wu8l_sb)):
                for pair in range(2):
                    nc.tensor.matmul(
                        hid[:],
                        lhsT=lhs8[:, pair, :, l * CHUNK:(l + 1) * CHUNK],
                        rhs=wu8[:, pair, :, :],
                        start=(i == 0),
                        stop=(i == 5),
                        perf_mode=mybir.MatmulPerfMode.DoubleRow,
                    )
                    i += 1
            th = work.tile([128, K], f16)
            nc.scalar.activation(out=th[:], in_=hid[:],
                                 func=mybir.ActivationFunctionType.Tanh)
            s_t = work.tile([128, 1], f32)
            nc.vector.scalar_tensor_tensor(
                out=scratch[:], in0=th[:], scalar=1.0,
                in1=brep_sb[:], op0=mybir.AluOpType.mult,
                op1=mybir.AluOpType.mult, accum_out=s_t[:])
            blk16 = work.tile([128, GROUP_ROWS], f16)
            nc.vector.tensor_scalar_mul(blk16[:], mask16_sb[:, cci, :], s_t[:])
            blk32 = work.tile([128, GROUP_ROWS], f32)
            nc.vector.tensor_scalar_mul(blk32[:], mask32_sb[:, cci, :], s_t[:])
            for h in range(2):
                nc.tensor.matmul(
                    ct_ps[:, h, 0:GROUP_ROWS],
                    lhsT=mtok[:, l, h * 128:(h + 1) * 128],
                    rhs=blk16[:],
                    start=(cci == 0),
                    stop=(cci == GROUP_CHUNKS - 1),
                )
            nc.tensor.matmul(
                s_ps[:],
                lhsT=blk32[:],
                rhs=ones_sb[:],
                start=(cci == 0),
                stop=(cci == GROUP_CHUNKS - 1),
            )
            if cci == GROUP_CHUNKS - 1:
                ct_sb = io_u.tile([128, 2, GROUP_ROWS], f32)
                nc.scalar.copy(out=ct_sb[:], in_=ct_ps[:, :, 0:GROUP_ROWS])
                s_sb = io_u.tile([GROUP_ROWS, 1], f32)
                nc.scalar.copy(out=s_sb[:], in_=s_ps[:])
                nc.sync.dma_start(out=ct_d[gg], in_=ct_sb[:])
                nc.sync.dma_start(
                    out=s_d[gg * GROUP_ROWS:(gg + 1) * GROUP_ROWS, :],
                    in_=s_sb[:])

    nc.compile()
    return nc


def host_constants(Wu: np.ndarray, b: np.ndarray):
    Wu = np.asarray(Wu, np.float32)
    b = np.asarray(b, np.float32)

    def pack_dr(a):  # [512, K] -> [p, pair, kt, k]
        return np.ascontiguousarray(a.reshape(2, 2, 128, -1).transpose(2, 0, 1, 3))

    wu_hi = Wu.astype(E4)
    wu_lo = (Wu - wu_hi.astype(np.float32)).astype(E4)
    brep = np.ascontiguousarray(
        np.broadcast_to(b.astype(np.float16), (128, K)))
    tp = np.arange(128)[:, None, None]
    ll = np.arange(GROUP_CHUNKS)[None, :, None]
    rr = np.arange(GROUP_ROWS)[None, None, :]
    mask = ((CHUNK * ll + tp) // N_TOK) == rr
    return {
        "wu8h": pack_dr(wu_hi),
        "wu8l": pack_dr(wu_lo),
        "brep": brep,
        "mask16": mask.astype(np.float16),
    }


def host_shard_arrays(m_sh: np.ndarray, p_sh: np.ndarray):
    """m_sh, p_sh: [T, 256] f32 token-major shard -> device input arrays."""
    mp = np.concatenate([m_sh, p_sh], axis=1)  # [T, 512]
    hi = mp.astype(E4)
    lo = (mp - hi.astype(np.float32)).astype(E4)

    def pack(a):  # [T, 512] -> [p, pair, kt, t]
        return np.ascontiguousarray(
            a.T.reshape(2, 2, 128, -1).transpose(2, 0, 1, 3))

    return {
        "mp8h": pack(hi),
        "mp8l": pack(lo),
        "m16": m_sh.astype(np.float16),
    }


def get_program(b_shard: int):
    if b_shard not in _prog_cache:
        _prog_cache[b_shard] = build_program(b_shard)
    return _prog_cache[b_shard]


def kernel(m: np.ndarray, p: np.ndarray, Wu: np.ndarray, b: np.ndarray
           ) -> np.ndarray:
    m = np.ascontiguousarray(np.asarray(m, np.float32))
    p = np.ascontiguousarray(np.asarray(p, np.float32))
    B = m.shape[0]
    assert B % N_CORES == 0
    b_shard = B // N_CORES

    nc = get_program(b_shard)
    consts = host_constants(Wu, b)

    mf = m.reshape(B * N_TOK, MD)
    pf = p.reshape(B * N_TOK, PD)
    tok_sh = b_shard * N_TOK
    in_maps = []
    for c in range(N_CORES):
        sh = host_shard_arrays(mf[c * tok_sh:(c + 1) * tok_sh],
                               pf[c * tok_sh:(c + 1) * tok_sh])
        in_maps.append({**sh, **consts})
    res = run_bass_kernel_spmd(nc, in_maps, list(range(N_CORES)))
    # ct: [n_groups, 128, 2, 64] per core; C[row, h*128+dp] = ct[g, dp, h, r]
    ct = np.concatenate([res.results[c]["ct"] for c in range(N_CORES)], axis=0)
    S_dev = np.concatenate(
        [res.results[c]["sv"] for c in range(N_CORES)], axis=0)[:, 0]
    C = ct.transpose(0, 3, 2, 1).reshape(B, MD)  # [G,dp,h,r]->[G,r,h,dp]
    with np.errstate(divide="ignore", over="ignore", invalid="ignore"):
        u = (C / S_dev[:, None]).astype(np.float32)

    # Host fixup: rows where |S| is small amplify device error via a = s/S;
    # recompute those exactly in fp64 from the original inputs.
    bad = ~np.isfinite(S_dev) | (np.abs(S_dev) < S_FIX_THRESH)
    bad |= ~np.isfinite(u).all(axis=1)
    if bad.any():
        idx = np.where(bad)[0]
        Wu64 = np.asarray(Wu, np.float64)
        b64 = np.asarray(b, np.float64)
        mp = np.concatenate([m[idx], p[idx]], axis=2).reshape(-1, D)
        th = np.tanh(mp.astype(np.float64) @ Wu64)
        s = (th @ b64).reshape(len(idx), N_TOK)
        S = s.sum(axis=1)
        Cx = np.einsum("bn,bnd->bd", s, m[idx].astype(np.float64))
        u[idx] = (Cx / S[:, None]).astype(np.float32)
    return u


# revision 32
# speedup vs baseline: 1.3108x; 1.0236x over previous
"""Trainium2 Bass kernel for ContentPopularityJointAttention.

Computes, for each batch row r:
    mp     = concat(m[r], p[r])            # (50, 512)
    hidden = tanh(mp @ Wu)                 # (50, 512)
    s      = hidden @ b                    # (50,)
    u[r]   = (sum_n s_n * m[r,n]) / (sum_n s_n)   # (256,)

Sharding: pure data parallel over the batch dim across 8 NeuronCores.

Per-core dataflow (512 rows = 25600 tokens, 128-token chunks):
  1. Host splits mp = concat(m,p) into fp8e4m3 hi + lo (hi = e4m3(mp),
     lo = e4m3(mp - hi); together ~9 mantissa bits) and pre-transposes
     both to the DoubleRow feature-major layout [128p, 2pair, 2kt, tok].
     Per macroblock (4,6,10,... chunks), three batched HWDGE DMAs on SP:
     hi, lo, and token-major fp16 m for pooling.
  2. Hidden matmul: 6 fp8 DoubleRow matmuls per chunk (3 passes
     hi*hi + lo*hi + hi*lo, each 2 pair-matmuls with 256-contraction)
     at 0.5 cycles/row -> PSUM [128tok, 512] fp32.  1536 PE cycles vs
     fp16's 2048; dropped lo*lo term ~2^-9 relative.
  3. ACT tanh -> fp16 SBUF.  4. DVE fused scalar_tensor_tensor:
     s = sum_k tanh*b (fp32 accumulate) -> [128,1].
  5. Pooling per chunk, transposed so the 64-row blk is the MOVING
     operand (2 matmuls of out-free 64 = 128 PE cycles):
     CT[d, row] += mtok_half^T @ (mask*s fp16); S[row] += (mask*s fp32)
     @ ones (exact fp32, 4 cycles).  Group end: ACT copies CT/S
     PSUM->SBUF, DMA out; u = C/S division happens on host.
  6. Host fixup: the sum-normalization a = s/S amplifies error by
     ~|C|/S^2; rows with |S| < 50 (~130 of 4096) are recomputed exactly
     on host in fp64.  Device-side fp8-split error (ds ~ 0.2 abs) is
     harmless for all other rows.
"""

import numpy as np
import ml_dtypes
from contextlib import ExitStack

import concourse.bass as bass
import concourse.bacc as bacc
import concourse.tile as tile
from concourse import mybir
from concourse.bass_utils import run_bass_kernel_spmd

N_CORES = 8
B_FULL, N_TOK, MD, PD = 4096, 50, 256, 256
D = MD + PD          # 512 contraction dim
K = 512              # hidden dim
CHUNK = 128          # tokens per chunk (partition dim)
GROUP_ROWS = 64      # batch rows per pooling PSUM accumulation group
GROUP_CHUNKS = GROUP_ROWS * N_TOK // CHUNK   # 25 (tiles the group exactly)
S_FIX_THRESH = 50.0  # host fixup threshold on |S|

f32 = mybir.dt.float32
f16 = mybir.dt.float16
f8 = mybir.dt.float8e4
E4 = ml_dtypes.float8_e4m3

_prog_cache: dict = {}


def _macroblocks(nchunks: int):
    """(start, nb) DMA macroblocks: small leading blocks to start PE early."""
    sizes = []
    for sz in (4, 6):
        if sum(sizes) + sz <= nchunks:
            sizes.append(sz)
    while (rem := nchunks - sum(sizes)) > 0:
        sizes.append(min(10, rem))
    out, c0 = [], 0
    for sz in sizes:
        out.append((c0, sz))
        c0 += sz
    return out


def build_program(b_shard: int):
    """Build the single-core Bass program (SPMD: same program, all cores)."""
    tokens = b_shard * N_TOK
    nchunks = tokens // CHUNK
    n_groups = b_shard // GROUP_ROWS
    assert b_shard % GROUP_ROWS == 0
    mblocks = dict(_macroblocks(nchunks))

    nc = bacc.Bacc("TRN2", target_bir_lowering=False, debug=False,
                   num_devices=N_CORES)

    # mp transposed fp8 hi/lo: [p, pair, kt, t] = mp[t, pair*256 + kt*128 + p]
    mp8h_d = nc.dram_tensor("mp8h", [128, 2, 2, tokens], f8,
                            kind="ExternalInput").ap()
    mp8l_d = nc.dram_tensor("mp8l", [128, 2, 2, tokens], f8,
                            kind="ExternalInput").ap()
    m8_d = nc.dram_tensor("m8", [tokens, MD], f8, kind="ExternalInput").ap()
    # Wu fp8 hi/lo: [p, pair, kt, k] = Wu[pair*256 + kt*128 + p, k]
    wu8h_d = nc.dram_tensor("wu8h", [128, 2, 2, K], f8, kind="ExternalInput").ap()
    wu8l_d = nc.dram_tensor("wu8l", [128, 2, 2, K], f8, kind="ExternalInput").ap()
    brep_d = nc.dram_tensor("brep", [128, K], f16, kind="ExternalInput").ap()
    mask16_d = nc.dram_tensor("mask16", [128, GROUP_CHUNKS, GROUP_ROWS], f16,
                              kind="ExternalInput").ap()
    ct_d = nc.dram_tensor("ct", [n_groups, 128, 2, GROUP_ROWS], f32,
                          kind="ExternalOutput").ap()
    s_d = nc.dram_tensor("sv", [b_shard, 1], f32, kind="ExternalOutput").ap()

    with tile.TileContext(nc) as tc, ExitStack() as ctx:
        singles = ctx.enter_context(tc.tile_pool(name="singles", bufs=1))
        io_t = ctx.enter_context(tc.tile_pool(name="ioT", bufs=2))
        io_m = ctx.enter_context(tc.tile_pool(name="ioM", bufs=2))
        io_u = ctx.enter_context(tc.tile_pool(name="ioU", bufs=2))
        work = ctx.enter_context(tc.tile_pool(name="work", bufs=3))
        psum_h = ctx.enter_context(tc.tile_pool(name="psumH", bufs=2, space="PSUM"))
        psum_u = ctx.enter_context(tc.tile_pool(name="psumU", bufs=2, space="PSUM"))
        psum_s = ctx.enter_context(tc.tile_pool(name="psumS", bufs=2, space="PSUM"))

        wu8h_sb = singles.tile([128, 2, 2, K], f8)
        wu8l_sb = singles.tile([128, 2, 2, K], f8)
        brep_sb = singles.tile([128, K], f16)
        mask16_sb = singles.tile([128, GROUP_CHUNKS, GROUP_ROWS], f16)
        mask32_sb = singles.tile([128, GROUP_CHUNKS, GROUP_ROWS], f32)
        ones_sb = singles.tile([128, 1], f32)
        scratch = singles.tile([128, K], f16)   # dead product of fused score op

        mp8h = mp8l = mtok = prev_mtok = None
        cur_nb = prev_nb = 0
        ct_ps = s_ps = None
        mb0 = 0

        for c in range(nchunks):
            if c in mblocks:
                nb = mblocks[c]
                mb0 = c
                t0 = c * CHUNK
                span = nb * CHUNK
                mp8h = io_t.tile([128, 2, 2, span], f8)
                nc.sync.dma_start(out=mp8h[:], in_=mp8h_d[:, :, :, t0:t0 + span])
                mp8l = io_t.tile([128, 2, 2, span], f8)
                nc.sync.dma_start(out=mp8l[:], in_=mp8l_d[:, :, :, t0:t0 + span])
                prev_mtok, prev_nb = mtok, cur_nb
                mtok = io_m.tile([128, nb, MD], f8)
                cur_nb = nb
                nc.sync.dma_start(
                    out=mtok[:],
                    in_=m8_d[t0:t0 + span, :].rearrange("(g p) d -> p g d", p=128))
                if c == 0:
                    nc.scalar.dma_start(out=wu8h_sb[:], in_=wu8h_d)
                    nc.scalar.dma_start(out=wu8l_sb[:], in_=wu8l_d)
                    nc.scalar.dma_start(out=brep_sb[:], in_=brep_d)
                    nc.scalar.dma_start(out=mask16_sb[:], in_=mask16_d)
                    nc.vector.memset(ones_sb[:], 1.0)
                    # fp32 masks derived on the otherwise-idle Pool engine
                    nc.gpsimd.tensor_copy(out=mask32_sb[:], in_=mask16_sb[:])

            l = c - mb0
            gg, cci = divmod(c, GROUP_CHUNKS)
            if cci == 0:
                # halves padded to separate PSUM banks (independent
                # accumulation groups must not share a zero region)
                ct_ps = psum_u.tile([128, 2, K], f32)
                s_ps = psum_s.tile([GROUP_ROWS, 1], f32)
            hid = psum_h.tile([128, K], f32)
            # hidden = mp @ Wu: 3-pass fp8 split, DoubleRow (256-contraction)
            i = 0
            for lhs8, wu8 in ((mp8h, wu8h_sb), (mp8l, wu8h_sb), (mp8h, wu8l_sb)):
                for pair in range(2):
                    nc.tensor.matmul(
                        hid[:],
                        lhsT=lhs8[:, pair, :, l * CHUNK:(l + 1) * CHUNK],
                        rhs=wu8[:, pair, :, :],
                        start=(i == 0),
                        stop=(i == 5),
                        perf_mode=mybir.MatmulPerfMode.DoubleRow,
                    )
                    i += 1
            th = work.tile([128, K], f16)
            nc.scalar.activation(out=th[:], in_=hid[:],
                                 func=mybir.ActivationFunctionType.Tanh)
            # scalar 1/16 keeps fp8-quantized scores inside e4m3 range; the
            # same scale lands on C and S so it cancels in u = C/S
            s_t = work.tile([128, 1], f32)
            nc.vector.scalar_tensor_tensor(
                out=scratch[:], in0=th[:], scalar=0.0625,
                in1=brep_sb[:], op0=mybir.AluOpType.mult,
                op1=mybir.AluOpType.mult, accum_out=s_t[:])
            solo = (cci == GROUP_CHUNKS - 1)  # 25 chunks: last one unpaired
            if cci % 2 == 0:
                pair8 = work.tile([128, 2, GROUP_ROWS], f8)
                if solo:
                    # pair slot 0 zeroed: partner slot reads chunk cci-1's
                    # mtok data but contributes nothing
                    nc.gpsimd.memset(pair8[:, 0, :], 0.0)
            nc.gpsimd.tensor_scalar_mul(pair8[:, 1 if solo else cci % 2, :],
                                        mask16_sb[:, cci, :], s_t[:])
            blk32 = work.tile([128, GROUP_ROWS], f32)
            nc.gpsimd.tensor_scalar_mul(blk32[:], mask32_sb[:, cci, :], s_t[:])
            if cci % 2 == 1 or solo:
                if cci % 2 == 1 and l == 0:
                    # pair straddles a DMA macroblock: two single-chunk fp8
                    # matmuls (prev chunk from the previous mtok tile)
                    for h in range(2):
                        nc.tensor.matmul(
                            ct_ps[:, h, 0:GROUP_ROWS],
                            lhsT=prev_mtok[:, prev_nb - 1, h * 128:(h + 1) * 128],
                            rhs=pair8[:, 0, :],
                            start=(cci == 1),
                            stop=False,
                        )
                        nc.tensor.matmul(
                            ct_ps[:, h, 0:GROUP_ROWS],
                            lhsT=mtok[:, 0, h * 128:(h + 1) * 128],
                            rhs=pair8[:, 1, :],
                            start=False,
                            stop=False,
                        )
                else:
                    # paired pooling: 256-token DoubleRow contraction over
                    # the two chunks' [tokens, d-half] slabs
                    for h in range(2):
                        nc.tensor.matmul(
                            ct_ps[:, h, 0:GROUP_ROWS],
                            lhsT=mtok[:, l - 1:l + 1, h * 128:(h + 1) * 128],
                            rhs=pair8[:],
                            start=(cci == 1),
                            stop=solo,
                            perf_mode=mybir.MatmulPerfMode.DoubleRow,
                        )
            nc.tensor.matmul(
                s_ps[:],
                lhsT=blk32[:],
                rhs=ones_sb[:],
                start=(cci == 0),
                stop=solo,
            )
            if cci == GROUP_CHUNKS - 1:
                ct_sb = io_u.tile([128, 2, GROUP_ROWS], f32)
                nc.scalar.copy(out=ct_sb[:], in_=ct_ps[:, :, 0:GROUP_ROWS])
                s_sb = io_u.tile([GROUP_ROWS, 1], f32)
                nc.scalar.copy(out=s_sb[:], in_=s_ps[:])
                nc.sync.dma_start(out=ct_d[gg], in_=ct_sb[:])
                nc.sync.dma_start(
                    out=s_d[gg * GROUP_ROWS:(gg + 1) * GROUP_ROWS, :],
                    in_=s_sb[:])

    nc.compile()
    return nc


def host_constants(Wu: np.ndarray, b: np.ndarray):
    Wu = np.asarray(Wu, np.float32)
    b = np.asarray(b, np.float32)

    def pack_dr(a):  # [512, K] -> [p, pair, kt, k]
        return np.ascontiguousarray(a.reshape(2, 2, 128, -1).transpose(2, 0, 1, 3))

    wu_hi = Wu.astype(E4)
    wu_lo = (Wu - wu_hi.astype(np.float32)).astype(E4)
    brep = np.ascontiguousarray(
        np.broadcast_to(b.astype(np.float16), (128, K)))
    tp = np.arange(128)[:, None, None]
    ll = np.arange(GROUP_CHUNKS)[None, :, None]
    rr = np.arange(GROUP_ROWS)[None, None, :]
    mask = ((CHUNK * ll + tp) // N_TOK) == rr
    return {
        "wu8h": pack_dr(wu_hi),
        "wu8l": pack_dr(wu_lo),
        "brep": brep,
        "mask16": mask.astype(np.float16),
    }


def host_shard_arrays(m_sh: np.ndarray, p_sh: np.ndarray):
    """m_sh, p_sh: [T, 256] f32 token-major shard -> device input arrays."""
    mp = np.concatenate([m_sh, p_sh], axis=1)  # [T, 512]
    hi = mp.astype(E4)
    lo = (mp - hi.astype(np.float32)).astype(E4)

    def pack(a):  # [T, 512] -> [p, pair, kt, t]
        return np.ascontiguousarray(
            a.T.reshape(2, 2, 128, -1).transpose(2, 0, 1, 3))

    return {
        "mp8h": pack(hi),
        "mp8l": pack(lo),
        "m8": m_sh.astype(E4),
    }


def get_program(b_shard: int):
    if b_shard not in _prog_cache:
        _prog_cache[b_shard] = build_program(b_shard)
    return _prog_cache[b_shard]


def kernel(m: np.ndarray, p: np.ndarray, Wu: np.ndarray, b: np.ndarray
           ) -> np.ndarray:
    m = np.ascontiguousarray(np.asarray(m, np.float32))
    p = np.ascontiguousarray(np.asarray(p, np.float32))
    B = m.shape[0]
    assert B % N_CORES == 0
    b_shard = B // N_CORES

    nc = get_program(b_shard)
    consts = host_constants(Wu, b)

    mf = m.reshape(B * N_TOK, MD)
    pf = p.reshape(B * N_TOK, PD)
    tok_sh = b_shard * N_TOK
    in_maps = []
    for c in range(N_CORES):
        sh = host_shard_arrays(mf[c * tok_sh:(c + 1) * tok_sh],
                               pf[c * tok_sh:(c + 1) * tok_sh])
        in_maps.append({**sh, **consts})
    res = run_bass_kernel_spmd(nc, in_maps, list(range(N_CORES)))
    # ct: [n_groups, 128, 2, 64] per core; C[row, h*128+dp] = ct[g, dp, h, r]
    ct = np.concatenate([res.results[c]["ct"] for c in range(N_CORES)], axis=0)
    S_dev = np.concatenate(
        [res.results[c]["sv"] for c in range(N_CORES)], axis=0)[:, 0]
    # both C and S carry the device-side 1/16 scaling, so u = C/S directly;
    # only the fixup threshold needs the true S = 16 * sv
    C = ct.transpose(0, 3, 2, 1).reshape(B, MD)  # [G,dp,h,r]->[G,r,h,dp]
    with np.errstate(divide="ignore", over="ignore", invalid="ignore"):
        u = (C / S_dev[:, None]).astype(np.float32)
    S_dev = S_dev * 16.0

    # Host fixup: rows where |S| is small amplify device error via a = s/S;
    # recompute those exactly in fp64 from the original inputs.
    bad = ~np.isfinite(S_dev) | (np.abs(S_dev) < S_FIX_THRESH)
    bad |= ~np.isfinite(u).all(axis=1)
    if bad.any():
        idx = np.where(bad)[0]
        Wu64 = np.asarray(Wu, np.float64)
        b64 = np.asarray(b, np.float64)
        mp = np.concatenate([m[idx], p[idx]], axis=2).reshape(-1, D)
        th = np.tanh(mp.astype(np.float64) @ Wu64)
        s = (th @ b64).reshape(len(idx), N_TOK)
        S = s.sum(axis=1)
        Cx = np.einsum("bn,bnd->bd", s, m[idx].astype(np.float64))
        u[idx] = (Cx / S[:, None]).astype(np.float32)
    return u
